# revision 11
# baseline (speedup 1.0000x reference)
"""ESMFold gated attention (B=8, Q=K=1024, C=256, H=8, DH=32) on 8 TRN2 NeuronCores.

Sharding: head-parallel. Core c computes head c of the attention for all 8
batches, then an 8-way AllToAll exchanges head-blocks for batch-blocks and
core c computes the output projection for batch c.

Device pipeline per core (layouts transposed host-side; no device transposes):
  1. Projections per batch: qT|kT|gT stacked in one PSUM ([96,Q] via
     rhs = x^T moving operand), sigmoid gate with fused bg bias on ACT;
     v natural [k,dh] with a ones-column appended (v_aug).
  2. Per batch: scores^T[k,q] = kT.T @ qT in PSUM (f32r, full-rate);
     softmax exp on ACT reads PSUM directly, bias_mask folded in free via
     ACT's per-partition bias; bias_pair folded by exp-factorization:
     attn = exp(s+mask) * E^T with E^T = exp(bias_pair^T) computed once
     (bf16; DVE 2x multiply). o_aug^T[33,q] = v_aug.T @ attn^T accumulated
     over k-tiles; row 32 = softmax denominators for free (ones column).
  3. Epilogue: reciprocal of denominators, partition-broadcast on GPSIMD,
     gate+normalize folded into two small DVE multiplies -> og^T[32,q].
  4. AllToAll over all 8 cores: chunk b of core c = head-c og for batch b;
     received chunks stack to ogT[256,q] for batch c in (head,dh) row order,
     matching Wo's rows. Output projection in natural [seq, C] layout (+bo).
"""

import math

import numpy as np

import concourse.bass as bass
import concourse.mybir as mybir
import concourse.tile as tile
F32 = mybir.dt.float32
F32R = mybir.dt.float32r
BF16 = mybir.dt.bfloat16

B, Q, K, C, H, DH = 8, 1024, 1024, 256, 8, 32
N_CORES = 8
KT = K // 128  # 8 k-tiles
INV_SQRT_DH = 1.0 / math.sqrt(DH)

def _split_multi_waits(nc):
    """The walrus build here allows at most one sem wait per instruction
    ("Too many sync wait commands"); move extra waits onto NoOps inserted
    just before, on the same engine (sequencers execute in order)."""
    ctr = 0
    for fn in nc.m.functions:
        for blk in fn.blocks:
            il = blk.instructions
            if not any(
                i.sync_info and i.sync_info.on_wait and len(i.sync_info.on_wait) > 1
                for i in il
            ):
                continue
            out = []
            for inst in il:
                si = inst.sync_info
                if si and si.on_wait and len(si.on_wait) > 1:
                    waits = list(si.on_wait)
                    for w in waits[:-1]:
                        ctr += 1
                        nop = mybir.InstNoOp(name=f"waitnop-{ctr}", ins=[], outs=[])
                        nop.engine = inst.engine
                        nop.sync_info = mybir.SyncInfo(on_wait=[w], on_update=[])
                        out.append(nop)
                    inst.sync_info = mybir.SyncInfo(
                        on_wait=[waits[-1]], on_update=list(si.on_update)
                    )
                out.append(inst)
            blk.instructions = out


def build_kernel() -> bass.Bass:
    nc = bass.Bass("TRN2", target_bir_lowering=False, debug=False, num_devices=N_CORES)

    # ---- per-core inputs (host pre-sharded / pre-transposed) ----
    xqT = nc.declare_dram_parameter("xqT", [B, C, Q], BF16, isOutput=False)
    xkvT = nc.declare_dram_parameter("xkvT", [B, C, K], BF16, isOutput=False)
    maskT = nc.declare_dram_parameter("maskT", [B, 128, KT], F32, isOutput=False)
    biasT = nc.declare_dram_parameter("biasT", [K, Q], F32, isOutput=False)
    wqg = nc.declare_dram_parameter("wqg", [2, 128, 64], BF16, isOutput=False)
    wk = nc.declare_dram_parameter("wk", [2, 128, DH], BF16, isOutput=False)
    wv = nc.declare_dram_parameter("wv", [2, 128, DH], BF16, isOutput=False)
    bgp = nc.declare_dram_parameter("bgp", [DH, 1], F32, isOutput=False)
    wo = nc.declare_dram_parameter("wo", [2, 128, C], BF16, isOutput=False)
    bor = nc.declare_dram_parameter("bor", [128, C], F32, isOutput=False)
    out = nc.declare_dram_parameter("out", [Q, C], F32, isOutput=True)


    with tile.TileContext(nc) as tc:
        with (
            tc.tile_pool(name="const", bufs=1) as const,
            tc.tile_pool(name="epool", bufs=1) as epool,
            tc.tile_pool(name="etmp", bufs=2) as etmp,
            tc.tile_pool(name="xin", bufs=2) as xin,
            tc.tile_pool(name="proj", bufs=2) as proj,
            tc.tile_pool(name="attn", bufs=3) as attnp,
            tc.tile_pool(name="epi", bufs=2) as epi,
            tc.tile_pool(name="ogp", bufs=1) as ogp,
            tc.tile_pool(name="fin", bufs=2) as finp,
            tc.tile_pool(name="ps_s", bufs=2, space="PSUM") as ps_s,
            tc.tile_pool(name="ps_o", bufs=1, space="PSUM") as ps_o,
            tc.tile_pool(name="ps_p", bufs=1, space="PSUM") as ps_p,
            tc.tile_pool(name="dram", bufs=1, space="DRAM") as dram,
        ):
            # ---- constants ----
            wqg_sb = const.tile([128, 2, 64], BF16)
            nc.sync.dma_start(wqg_sb[:], wqg.rearrange("t p m -> p t m"))
            wk_sb = const.tile([128, 2, DH], BF16)
            nc.sync.dma_start(wk_sb[:], wk.rearrange("t p m -> p t m"))
            wv_sb = const.tile([128, 2, DH], BF16)
            nc.sync.dma_start(wv_sb[:], wv.rearrange("t p m -> p t m"))
            bg_sb = const.tile([DH, 1], F32)
            nc.sync.dma_start(bg_sb[:], bgp[:])
            wo_sb = const.tile([128, 2, C], BF16)
            nc.sync.dma_start(wo_sb[:], wo.rearrange("t p m -> p t m"))
            bo_sb = const.tile([128, C], F32)
            nc.sync.dma_start(bo_sb[:], bor[:])
            mask_sb = const.tile([128, B, KT], F32)
            nc.sync.dma_start(mask_sb[:], maskT.rearrange("b p j -> p b j"))
            ones_sb = const.tile([1, DH], F32)
            nc.vector.memset(ones_sb[:], 1.0)

            # ---- E^T = exp(bias_pair^T) for this head, bf16, once ----
            e_sb = epool.tile([128, KT, Q], BF16)
            for j0 in range(0, KT, 4):
                t = etmp.tile([128, 4, Q], F32, tag="etmp")
                nc.sync.dma_start(
                    t[:], biasT[j0 * 128:(j0 + 4) * 128, :].rearrange(
                        "(jj p) q -> p jj q", p=128
                    )
                )
                nc.scalar.activation(
                    e_sb[:, j0:j0 + 4, :], t[:],
                    mybir.ActivationFunctionType.Exp,
                )

            og_sb = ogp.tile([DH, B, Q], BF16)

            for b in range(B):
                # ---- projections for batch b ----
                xq_sb = xin.tile([128, 2, Q], BF16, tag="xq")
                nc.sync.dma_start(xq_sb[:], xqT[b].rearrange("(t p) q -> p t q", p=128))
                xkv_sb = xin.tile([128, 2, K], BF16, tag="xkv")
                nc.sync.dma_start(xkv_sb[:], xkvT[b].rearrange("(t p) q -> p t q", p=128))

                # qT|gT: [64 rows = q(32) | g(32), Q] from q_x
                qg_ps = ps_p.tile([64, Q], F32, tag="pp")
                for ch in range(2):
                    for ct in range(2):
                        nc.tensor.matmul(
                            qg_ps[:, ch * 512:(ch + 1) * 512],
                            lhsT=wqg_sb[:, ct, :],
                            rhs=xq_sb[:, ct, ch * 512:(ch + 1) * 512],
                            start=(ct == 0), stop=(ct == 1),
                        )
                q_sb = proj.tile([DH, Q], BF16, tag="q")
                nc.vector.tensor_scalar_mul(q_sb[:], qg_ps[0:DH, :], INV_SQRT_DH)
                g_sb = proj.tile([DH, Q], F32, tag="g")
                nc.scalar.activation(
                    g_sb[:], qg_ps[DH:2 * DH, :],
                    mybir.ActivationFunctionType.Sigmoid,
                    bias=bg_sb[:, 0:1],
                )
                # kT: [32, K] from kv_x
                k_ps = ps_p.tile([DH, K], F32, tag="pp")
                for ch in range(2):
                    for ct in range(2):
                        nc.tensor.matmul(
                            k_ps[:, ch * 512:(ch + 1) * 512],
                            lhsT=wk_sb[:, ct, :],
                            rhs=xkv_sb[:, ct, ch * 512:(ch + 1) * 512],
                            start=(ct == 0), stop=(ct == 1),
                        )
                k_sb = proj.tile([DH, K], BF16, tag="k")
                nc.vector.tensor_copy(k_sb[:], k_ps[:])

                # v natural [k, dh] + ones column, bf16
                v_sb = proj.tile([128, KT, DH + 1], BF16, tag="v")
                nc.gpsimd.memset(v_sb[:, :, DH:DH + 1], 1.0)
                for j in range(KT):
                    v_ps = ps_p.tile([128, DH], F32, tag="pp")
                    for ct in range(2):
                        nc.tensor.matmul(
                            v_ps[:],
                            lhsT=xkv_sb[:, ct, j * 128:(j + 1) * 128],
                            rhs=wv_sb[:, ct, :],
                            start=(ct == 0), stop=(ct == 1),
                        )
                    nc.vector.tensor_copy(v_sb[:, j, 0:DH], v_ps[:])

                # ---- attention for (batch b, this core's head) ----
                o_ps = ps_o.tile([DH + 1, Q], F32, tag="o")
                for j in range(KT):
                    s_ps = ps_s.tile([128, Q], F32, tag="s")
                    for ch in range(2):
                        nc.tensor.matmul(
                            s_ps[:, ch * 512:(ch + 1) * 512],
                            lhsT=k_sb[:, j * 128:(j + 1) * 128],
                            rhs=q_sb[:, ch * 512:(ch + 1) * 512],
                            start=True, stop=True,
                        )
                    at = attnp.tile([128, Q], BF16, tag="at")
                    nc.scalar.activation(
                        at[:], s_ps[:], mybir.ActivationFunctionType.Exp,
                        bias=mask_sb[:, b, j:j + 1],
                    )
                    at2 = attnp.tile([128, Q], BF16, tag="at2")
                    nc.vector.tensor_mul(at2[:], at[:], e_sb[:, j, :])
                    for ch in range(2):
                        nc.tensor.matmul(
                            o_ps[:, ch * 512:(ch + 1) * 512],
                            lhsT=v_sb[:, j, :],
                            rhs=at2[:, ch * 512:(ch + 1) * 512],
                            start=(j == 0), stop=(j == KT - 1),
                        )
                # epilogue: og^T = o^T * bcast(1/sum) * gT
                r_sb = epi.tile([1, Q], F32, tag="r")
                nc.vector.reciprocal(r_sb[:], o_ps[DH:DH + 1, :])
                rb_ps = ps_s.tile([DH, Q], F32, tag="s")
                for ch in range(2):
                    nc.tensor.matmul(
                        rb_ps[:, ch * 512:(ch + 1) * 512],
                        lhsT=ones_sb[:],
                        rhs=r_sb[:, ch * 512:(ch + 1) * 512],
                        start=True, stop=True,
                    )
                gr_sb = epi.tile([DH, Q], F32, tag="gr")
                nc.vector.tensor_mul(gr_sb[:], g_sb[:], rb_ps[:])
                nc.vector.tensor_mul(og_sb[:, b, :], o_ps[0:DH, :], gr_sb[:])

            # ---- exchange + output projection ----
            a2a_in = dram.tile([B, DH, Q], BF16)
            a2a_out = dram.tile([B, DH, Q], BF16)
            nc.sync.dma_start(a2a_in.rearrange("b r q -> r b q"), og_sb[:])
            nc.gpsimd.collective_compute(
                "AllToAll",
                mybir.AluOpType.bypass,
                replica_groups=[list(range(N_CORES))],
                ins=[a2a_in.opt()],
                outs=[a2a_out.opt()],
            )
            # received: chunk h = og^T[32, Q] of head h for my batch
            ogT_sb = finp.tile([128, 2, Q], BF16, tag="ogT")
            nc.sync.dma_start(
                ogT_sb[:],
                a2a_out.rearrange("(ct hh) w q -> (hh w) ct q", ct=2),
            )
            for s in range(Q // 128):
                out_ps = ps_s.tile([128, C], F32, tag="s")
                for ct in range(2):
                    nc.tensor.matmul(
                        out_ps[:],
                        lhsT=ogT_sb[:, ct, s * 128:(s + 1) * 128],
                        rhs=wo_sb[:, ct, :],
                        start=(ct == 0), stop=(ct == 1),
                    )
                out_sb = finp.tile([128, C], F32, tag="outsb")
                nc.vector.tensor_add(out_sb[:], out_ps[:], bo_sb[:])
                nc.sync.dma_start(out[s * 128:(s + 1) * 128, :], out_sb[:])

    _split_multi_waits(nc)
    return nc


def shard_inputs(q_x, kv_x, bias_mask, bias_pair, Wq, Wk, Wv, Wg, bg, Wo, bo):
    """Build the per-core input maps (host-side slicing/layout only)."""
    q_x = np.ascontiguousarray(q_x, np.float32)
    kv_x = np.ascontiguousarray(kv_x, np.float32)
    bias_mask = np.asarray(bias_mask, np.float32)
    bias_pair = np.asarray(bias_pair, np.float32)
    Wq, Wk, Wv, Wg = (np.asarray(w, np.float32) for w in (Wq, Wk, Wv, Wg))
    import ml_dtypes
    bf16 = ml_dtypes.bfloat16
    xqT = np.ascontiguousarray(q_x.transpose(0, 2, 1).astype(bf16))
    xkvT = np.ascontiguousarray(kv_x.transpose(0, 2, 1).astype(bf16))
    maskT_all = np.ascontiguousarray(
        bias_mask[:, 0, 0, :].reshape(B, KT, 128).transpose(0, 2, 1)
    )
    import ml_dtypes
    wo_full = np.ascontiguousarray(np.asarray(Wo, np.float32).reshape(2, 128, C).astype(ml_dtypes.bfloat16))
    bo_rep = np.ascontiguousarray(np.broadcast_to(np.asarray(bo, np.float32), (128, C)))
    in_maps = []
    for c in range(N_CORES):
        hs = slice(c * DH, (c + 1) * DH)
        in_maps.append({
            "xqT": xqT,
            "xkvT": xkvT,
            "maskT": maskT_all,
            "biasT": np.ascontiguousarray(bias_pair[0, c].T),
            "wqg": np.ascontiguousarray(
                np.concatenate([Wq[:, hs], Wg[:, hs]], axis=1)
                .reshape(2, 128, 64).astype(bf16)
            ),
            "wk": np.ascontiguousarray(Wk[:, hs].reshape(2, 128, DH).astype(bf16)),
            "wv": np.ascontiguousarray(Wv[:, hs].reshape(2, 128, DH).astype(bf16)),
            "bgp": np.ascontiguousarray(np.asarray(bg, np.float32)[hs].reshape(DH, 1)),
            "wo": wo_full,
            "bor": bo_rep,
        })
    return in_maps


def assemble_output(results):
    out = np.empty((B, Q, C), np.float32)
    for c in range(N_CORES):
        out[c] = results[c]["out"]
    return out


_NC_CACHE = None


def kernel(**inputs) -> np.ndarray:
    global _NC_CACHE
    from concourse.bass_utils import run_bass_kernel_spmd

    if _NC_CACHE is None:
        _NC_CACHE = build_kernel()
    in_maps = shard_inputs(**inputs)
    res = run_bass_kernel_spmd(_NC_CACHE, in_maps, list(range(N_CORES)))
    return assemble_output(res.results)
